# revision 78
# baseline (speedup 1.0000x reference)
"""Distributed causal self-attention kernel for one TRN2 chip (8 NeuronCores).

Problem: y = CausalSelfAttention(x) with B=2, T=2048, C=1024, 16 heads x 64.

Sharding (per core c = b*4 + hg;  b = batch, hg = head-group of 4 heads):
  - Q/K/V projections: column-sharded per head group (each core computes its
    4 heads' Q,K,V from the full x of its batch).
  - Attention: fully local (4 heads per core), flash-style. Scores are kept
    transposed (s^T[k, q]); the AV matmul is emitted in the *output-
    transposed* orientation (out y[q, d], lhsT = e-slice, rhs = V) so each
    AV streams only 65 columns instead of 512 -- half the PE time.
  - Row-sums for softmax ride the AV matmul as a 65th "ones" column of V.
  - y[q, d] is normalized on DVE, transposed back to y^T[d, q] with the DMA
    xbar (dma_start_transpose), and AllGathered per (pair, 512-q-tile):
    8 small collectives spread across the timeline instead of 4 big ones.
  - o_proj: each core computes its own 256 output columns from the gathered
    y^T -> output shards are disjoint; the host just concatenates.

Scheduling: PE executes in issue order, and the attention inner loop is
Activation-bound (exp on [128,1024] tiles ~1us vs ~0.65us of PE work per
k-block). The emitter therefore software-pipelines exp one k-block ahead
and weaves "filler" matmul units (the other pair's projections, o_proj
tiles) into the attention stream, guided by an estimated-time clock that
knows DMA arrival and collective completion times. Input DMA is ordered by
first use (wq, x t-tile 0, wk, masks, ...) so attention starts ~3us in.

All matmuls run in bf16 (fp32 accumulation in PSUM); inputs are converted
to bf16 on the host. QK^T matmuls (contraction dim 64) are packed
two-per-PE via tile_position row tiling.
"""
import sys
sys.path.insert(0, '/opt/trn_rl_repo')
import numpy as np
import ml_dtypes

B, T, C = 2, 2048, 1024
NH, HD = 16, 64
N_CORES = 8
GROUPS = [[0, 1, 2, 3], [4, 5, 6, 7]]
HPC = NH // 4            # heads per core = 4
SH = HPC * HD            # per-core projection width = 256
NCB = C // 128           # contraction blocks = 8
QT = 512                 # query tile
VW = HPC * 65            # vhat row width = 260
BF16 = ml_dtypes.bfloat16

PE_NS = 1.0 / 2.4        # ns per PE column-cycle at full clock
EXP_NS = 1000.0          # Act time for one [128,1024] exp tile
DMA_BPS = 320e9          # effective DMA_ENGINES bandwidth for estimates

_CACHE = {}


def _build(t_len):
    import concourse.bass as bass
    import concourse.bacc as bacc
    import concourse.tile as tile
    import concourse.mybir as mybir
    dt = mybir.dt
    f32, bf16 = dt.float32, dt.bfloat16

    nqt = t_len // QT        # query tiles per pair = 4
    ntc = t_len // 128       # t chunks of 128 = 16

    nc = bacc.Bacc("TRN2", target_bir_lowering=False, debug=False,
                   num_devices=N_CORES)
    # inputs arrive pre-blocked on the host: [(cblk p) ...] -> [p, cblk*...]
    xT = nc.dram_tensor("xT", [128, NCB * t_len], bf16, kind="ExternalInput")
    wq = nc.dram_tensor("wqT", [128, NCB * SH], bf16, kind="ExternalInput")
    wk = nc.dram_tensor("wkT", [128, NCB * SH], bf16, kind="ExternalInput")
    wv = nc.dram_tensor("wvT", [128, NCB * SH], bf16, kind="ExternalInput")
    wo = nc.dram_tensor("woT", [128, NCB * SH], bf16, kind="ExternalInput")
    masks = nc.dram_tensor("masks", [128, 1024], bf16, kind="ExternalInput")
    out = nc.dram_tensor("out", [SH, t_len], bf16, kind="ExternalOutput")

    with tile.TileContext(nc) as tc:
        with tc.tile_pool(name="big", bufs=1) as big, \
             tc.tile_pool(name="epool", bufs=4) as epool, \
             tc.tile_pool(name="ytp", bufs=3) as ytp, \
             tc.tile_pool(name="yttp", bufs=3) as yttp, \
             tc.tile_pool(name="rpool", bufs=4) as rpool, \
             tc.tile_pool(name="ygp", bufs=8) as ygp, \
             tc.tile_pool(name="stp", bufs=3) as stp, \
             tc.tile_pool(name="qkps", bufs=2, space="PSUM") as qkps, \
             tc.tile_pool(name="yps", bufs=1, space="PSUM") as yps, \
             tc.tile_pool(name="fps", bufs=2, space="PSUM") as fps, \
             tc.tile_pool(name="dram", bufs=1, space="DRAM") as dram:

            # ---- resident SBUF tensors ----
            xt = big.tile([128, NCB * t_len], bf16)       # x^T, c-blocked
            wq_sb = big.tile([128, NCB * SH], bf16)
            wk_sb = big.tile([128, NCB * SH], bf16)
            wv_sb = big.tile([128, NCB * SH], bf16)
            wo_sb = big.tile([128, NCB * SH], bf16)
            mask_sb = big.tile([128, 1024], bf16)
            qt_sb = big.tile([128, 2 * t_len], bf16)      # Q^T, pair-blocked
            kt_sb = big.tile([128, 2 * t_len], bf16)
            vhat_sb = big.tile([128, ntc * VW], bf16)     # [V_h | 1] per head

            # ones column of vhat (col 64 of each head's 65-wide slot)
            nc.gpsimd.memset(
                vhat_sb.rearrange("p (t h c) -> p t h c", h=HPC, c=65)[:, :, :, 64:65],
                1.0)

            # ---- input DMA: few large transfers, ordered by first consumer ----
            # Estimated arrival clock for the scheduler.
            dma_cum = [0.0]

            def stage(dst, src, nbytes):
                nc.sync.dma_start(dst, src)
                dma_cum[0] += nbytes
                return 2000.0 + dma_cum[0] / DMA_BPS * 1e9

            def stage_x(tq, kblk=None):
                kb = slice(None) if kblk is None else slice(*kblk)
                nk = NCB if kblk is None else kblk[1] - kblk[0]
                sl = (slice(None), kb, slice(tq * QT, (tq + 1) * QT))
                return stage(
                    xt.rearrange("p (k t) -> p k t", k=NCB)[sl],
                    xT.rearrange("p (k t) -> p k t", k=NCB)[sl],
                    128 * nk * QT * 2)

            def stage_wh(w_sb, w_in, pair, k0=0):
                """one pair's 128-column half of a weight tensor, strided.
                256B elements pay the sub-512B descriptor penalty -> bill 2x."""
                sl = (slice(None), slice(k0, None), slice(pair * 128, (pair + 1) * 128))
                return stage(
                    w_sb.rearrange("p (k c) -> p k c", k=NCB)[sl],
                    w_in.rearrange("p (k c) -> p k c", k=NCB)[sl],
                    128 * (NCB - k0) * 128 * 2 * 2)

            WB = 128 * NCB * SH * 2    # bytes of one full weight tensor
            x_ready = [0.0] * nqt
            # tiny first chunks so the very first matmul starts ~2us earlier
            stage(wq_sb[:, 0:128], wq[:, 0:128], 128 * 128 * 2)
            stage(xt[:, 0:QT], xT[:, 0:QT], 128 * QT * 2)
            wq_ready = stage_wh(wq_sb, wq, 0, k0=1)
            stage_x(0, (1, 4))
            x_ready[0] = stage_x(0, (4, 8))
            wk_ready = stage_wh(wk_sb, wk, 0)
            mask_ready = stage(mask_sb[:], masks[:], 128 * 1024 * 2)
            wv_ready = stage_wh(wv_sb, wv, 0)
            x_ready[1] = stage_x(1)
            wq1_ready = stage_wh(wq_sb, wq, 1)
            wk1_ready = stage_wh(wk_sb, wk, 1)
            x_ready[2] = stage_x(2)
            wv1_ready = stage_wh(wv_sb, wv, 1)
            x_ready[3] = stage_x(3)
            wo_ready = stage(wo_sb[:], wo[:], WB)
            wk0_ready = wk_ready
            wv0_ready = wv_ready

            # ---- DRAM bounce buffers ----
            # pair 0: one AllGather per q-tile (fine overlap with attn(0)).
            # pair 1: q-tiles 0+1 share one gather -- less COLLECTIVE_CORES
            # backlog in the contended end-of-kernel window.
            # ship_plan[(pair, qi)] = (in_tile, out_tile, col_off, fire, qis)
            ship_plan = {}
            for q in range(nqt):
                i_t = dram.tile([128, QT], bf16, name=f"agin0{q}")
                o_t = dram.tile([512, QT], bf16, name=f"agout0{q}")
                ship_plan[(0, q)] = (i_t, o_t, 0, True, [(0, q)])
            ag1a_i = dram.tile([128, 2 * QT], bf16, name="agin1a")
            ag1a_o = dram.tile([512, 2 * QT], bf16, name="agout1a")
            ship_plan[(1, 0)] = (ag1a_i, ag1a_o, 0, False, [])
            ship_plan[(1, 1)] = (ag1a_i, ag1a_o, QT, True, [(1, 0), (1, 1)])
            dummy_dr = dram.tile([128, 8], bf16, name="dummy_dr")
            dummy_sb = ytp.tile([128, 8], bf16, name="dummy_sb", bufs=1,
                                tag="dummy")
            for q in (2, 3):
                i_t = dram.tile([128, QT], bf16, name=f"agin1{q}")
                o_t = dram.tile([512, QT], bf16, name=f"agout1{q}")
                ship_plan[(1, q)] = (i_t, o_t, 0, True, [(1, q)])

            # ================= scheduler machinery =================
            # est[0] = PE-work cursor (ns), est[1] = Act cursor (ns).
            est = [2000.0, 0.0]

            def pe(cols):
                est[0] += cols * PE_NS

            def act_tile():
                est[1] = max(est[1], est[0]) + EXP_NS

            class Unit:
                """A filler work unit: generator emitting matmuls lazily."""

                def __init__(self, key, ready, gen, cols_per_step):
                    self.key, self.ready, self.gen = key, ready, gen
                    self.cols = cols_per_step
                    self.done = False

                def step(self):
                    try:
                        next(self.gen)
                        pe(self.cols)
                        return True
                    except StopIteration:
                        self.done = True
                        return False

            units = []          # ordered list of Units
            by_key = {}

            def add_unit(key, ready, gen, cols):
                u = Unit(key, ready, gen, cols)
                units.append(u)
                by_key[key] = u

            # deferred emission events (e.g. agout pulls): emitted once the
            # est clock passes `ready`, so their sem waits resolve quickly
            # and never park long on an engine SEQ.
            events = []

            def at_time(ready, fn, key=None):
                events.append([ready, fn, False, key])

            def pump():
                now = max(est[0], est[1])
                for ev in events:
                    if not ev[2] and now >= ev[0]:
                        ev[1]()
                        ev[2] = True

            def pump_force(key):
                for ev in events:
                    if not ev[2] and ev[3] == key:
                        ev[1]()
                        ev[2] = True

            active = [None]

            def _next_active():
                if active[0] is not None and not active[0].done:
                    return active[0]
                now = max(est[0], est[1])
                for u in units:
                    if not u.done and u.ready <= now:
                        active[0] = u
                        return u
                return None

            def fill(budget_ns):
                while budget_ns > 0:
                    u = _next_active()
                    if u is None or not u.step():
                        if u is None:
                            return
                        continue
                    budget_ns -= u.cols * PE_NS

            def force(key):
                u = by_key[key]
                if u.done:
                    return
                est[0] = max(est[0], u.ready)
                while u.step():
                    pass

            # ================= work-unit generators =================
            def qk_proj_gen(pair, w_sb, dst_sb, n):
                """Q^T/K^T projection tile n for one pair: 8 matmuls + copy."""
                ps = fps.tile([128, QT], f32, name="fp", tag="fp")
                for k in range(NCB):
                    nc.tensor.matmul(
                        ps[:],
                        lhsT=w_sb[:, k * SH + pair * 128: k * SH + (pair + 1) * 128],
                        rhs=xt[:, k * t_len + n * QT: k * t_len + (n + 1) * QT],
                        start=(k == 0), stop=(k == NCB - 1))
                    yield
                nc.vector.tensor_copy(
                    dst_sb[:, pair * t_len + n * QT: pair * t_len + (n + 1) * QT],
                    ps[:])

            def v_proj_gen(pair, tch):
                """V (2 heads) for t-chunk tch, written into vhat slots."""
                ps = fps.tile([128, 128], f32, name="fp", tag="fp")
                for k in range(NCB):
                    nc.tensor.matmul(
                        ps[:],
                        lhsT=xt[:, k * t_len + tch * 128: k * t_len + (tch + 1) * 128],
                        rhs=wv_sb[:, k * SH + pair * 128: k * SH + (pair + 1) * 128],
                        start=(k == 0), stop=(k == NCB - 1))
                    yield
                dst = vhat_sb.rearrange("p (t h c) -> p t h c", h=HPC, c=65)[
                    :, tch, 2 * pair: 2 * pair + 2, 0:64]
                nc.vector.tensor_copy(
                    dst, ps.rearrange("p (h c) -> p h c", h=2))

            ygs = {}            # (pair, qi) -> (pulled tile, group_len, off)
            pend_pull = [None]  # (sbuf tile, dram out tile, group_len)

            def flush_pull():
                if pend_pull[0] is not None:
                    t_, o_, gl_ = pend_pull[0]
                    # split merged-group pulls per 512-q-tile: the first
                    # o_proj tile only needs the first half, ~1.5us sooner
                    nsp = gl_ // QT
                    for sp in range(nsp):
                        nc.gpsimd.dma_start(
                            t_[:].rearrange("p (r s t) -> p r s t",
                                            r=4, s=nsp)[:, :, sp],
                            o_.rearrange("(r p) (s t) -> p r s t",
                                         r=4, s=nsp)[:, :, sp],
                        )
                    pend_pull[0] = None

            def o_proj_gen(tq):
                """o_proj for t-tile tq: 16 matmuls from pulled y^T, store."""
                def yg(cb):
                    r, p2 = divmod(cb, 2)
                    t_, gl_, off_ = ygs[(p2, tq)]
                    return t_[:, r * gl_ + off_: r * gl_ + off_ + QT]
                st = stp.tile([128, 2 * QT], bf16, name="st")
                for m in range(2):
                    ps = fps.tile([128, QT], f32, name="fp", tag="fp")
                    for cb in range(NCB):
                        nc.tensor.matmul(
                            ps[:],
                            lhsT=wo_sb[:, cb * SH + m * 128: cb * SH + (m + 1) * 128],
                            rhs=yg(cb),
                            start=(cb == 0), stop=(cb == NCB - 1))
                        yield
                    nc.vector.tensor_copy(st[:, m * QT:(m + 1) * QT], ps[:])
                    # per-m store: m0 ships while m1 still accumulates
                    nc.sync.dma_start(
                        out[m * 128:(m + 1) * 128, tq * QT:(tq + 1) * QT],
                        st[:, m * QT:(m + 1) * QT])

            # register projection units
            for n in range(nqt):
                add_unit(('q0', n), x_ready[n],
                         qk_proj_gen(0, wq_sb, qt_sb, n), QT)
                add_unit(('k0', n), max(x_ready[n], wk0_ready),
                         qk_proj_gen(0, wk_sb, kt_sb, n), QT)
            for c in range(ntc):
                add_unit(('v0', c), max(x_ready[c // 4], wv0_ready),
                         v_proj_gen(0, c), 128)
            for n in range(nqt):
                add_unit(('q1', n), max(x_ready[n], wq1_ready),
                         qk_proj_gen(1, wq_sb, qt_sb, n), QT)
                add_unit(('k1', n), max(x_ready[n], wk1_ready),
                         qk_proj_gen(1, wk_sb, kt_sb, n), QT)
            for c in range(ntc):
                add_unit(('v1', c), max(x_ready[c // 4], wv1_ready),
                         v_proj_gen(1, c), 128)

            coll_done = {}      # (pair, qi) -> est completion ns

            # ================= attention =================
            def attention_qi(pair, qi, extra=(), pre_diag=None):
                """extra: list of (frac, kind, key) actions fired when the
                emission reaches `frac` of this qi's exp tiles. kind 'pump'
                force-emits a deferred event; kind 'unit' unlocks a filler
                unit (pumping its pulls first)."""
                g = 4 * qi
                q0 = qi * QT
                total_tiles = g + 3
                tcount = [0]
                pend_extra = list(extra)

                def poll_extra():
                    frac = tcount[0] / total_tiles
                    for ex in list(pend_extra):
                        if frac >= ex[0]:
                            if ex[1] == 'pump':
                                pump_force(ex[2])
                            else:
                                flush_pull()
                                if ex[2] in by_key:
                                    by_key[ex[2]].ready = 0.0
                            pend_extra.remove(ex)

                def qk_mm(dst, kb, qa, w, h01):
                    nc.tensor.matmul(
                        dst,
                        lhsT=kt_sb[h01 * 64:(h01 + 1) * 64,
                                   pair * t_len + kb * 128: pair * t_len + (kb + 1) * 128],
                        rhs=qt_sb[h01 * 64:(h01 + 1) * 64,
                                  pair * t_len + qa: pair * t_len + qa + w],
                        start=True, stop=True,
                        tile_position=(h01 * 64, 0))
                    pe(w)

                Yab = yps.tile([128, 260], f32, name="Yab", tag="Yab")
                Ycd = yps.tile([128, 260], f32, name="Ycd", tag="Ycd")
                Y = {0: (Yab, 0), 1: (Yab, 1), 2: (Ycd, 0), 3: (Ycd, 1)}
                started = set()
                stop_at = {0: ('b0', 0), 1: ('b0', 1),
                           2: ('b1', 0), 3: ('b1', 1)}

                def av_mm(e_tile, ecol, j, h01, kb, tag):
                    yt_, jj = Y[j]
                    # ONE start=True per Y tile per round: start marks the
                    # whole PSUM zero-region (bank) pending-zero, so each
                    # slot's first write then overwrites and later writes
                    # accumulate. A second start in the same bank would
                    # re-poison already-written slots.
                    key = id(yt_)
                    st_ = key not in started
                    started.add(key)
                    nc.tensor.matmul(
                        yt_[:, jj * 130 + h01 * 65: jj * 130 + (h01 + 1) * 65],
                        lhsT=e_tile[:, ecol: ecol + 128],
                        rhs=vhat_sb[:, kb * VW + (2 * pair + h01) * 65:
                                    kb * VW + (2 * pair + h01 + 1) * 65],
                        start=st_, stop=(stop_at[j] == tag),
                        skip_group_check=True)
                    pe(65)

                pend = None     # deferred AV list from the previous tile

                def flush_pend():
                    nonlocal pend
                    if pend is not None:
                        est[0] = max(est[0], pend[0])
                        for f in pend[1]:
                            f()
                        pend = None

                def stage_tile(e_tile, avs):
                    nonlocal pend
                    flush_pend()
                    tcount[0] += 1
                    poll_extra()
                    pump()
                    fill(max(0.0, (est[1] - est[0]) - 600.0))
                    pend = (est[1], [])
                    for a in avs:
                        pend[1].append(a)

                # full k-blocks
                for kb in range(g):
                    qk = qkps.tile([128, 1024], f32, name="qk", tag="qk")
                    for h01 in (0, 1):
                        qk_mm(qk[:, h01 * 512:(h01 + 1) * 512], kb, q0, 512, h01)
                    e = epool.tile([128, 1024], bf16, name="e")
                    nc.scalar.activation(e[:], qk[:],
                                         mybir.ActivationFunctionType.Exp,
                                         scale=1.0 / np.sqrt(HD))
                    act_tile()
                    avs = []
                    for h01 in (0, 1):
                        for j in range(4):
                            avs.append(
                                (lambda e_=e, h_=h01, j_=j, kb_=kb:
                                 av_mm(e_, h_ * 512 + j_ * 128, j_, h_, kb_,
                                       ('full', kb_))))
                    stage_tile(e, avs)

                # K tile qi / V chunks 4qi..4qi+3 are first needed here; a
                # late force keeps the head of the q-tile exp-dense
                if pre_diag is not None:
                    pre_diag()

                # diagonal: mid supertile (kb g,g+1 vs upper q-half, unmasked)
                mid = qkps.tile([128, 1024], f32, name="qk", tag="qk")
                for i in (0, 1):
                    for h01 in (0, 1):
                        qk_mm(mid[:, (h01 * 2 + i) * 256:(h01 * 2 + i + 1) * 256],
                              g + i, q0 + 256, 256, h01)
                em = epool.tile([128, 1024], bf16, name="e")
                nc.scalar.activation(em[:], mid[:],
                                     mybir.ActivationFunctionType.Exp,
                                     scale=1.0 / np.sqrt(HD))
                act_tile()
                avs = []
                for h01 in (0, 1):
                    for i in (0, 1):
                        for jj in (0, 1):   # j = 2 + jj
                            avs.append(
                                (lambda e_=em, h_=h01, i_=i, jj_=jj:
                                 av_mm(e_, (h_ * 2 + i_) * 256 + jj_ * 128,
                                       2 + jj_, h_, g + i_, ('mid', i_))))
                stage_tile(em, avs)

                # diagonal bands (masked): band u covers q-half u vs kb g+2u+{0,1}
                # band order (1, 0): Ycd (q-blocks 2,3) closes one tile early
                # so its normalize/ship overlaps band0's compute.
                yt = ytp.tile([128, QT], bf16, name="yt")
                ytT = yttp.tile([128, QT], bf16, name="ytT")

                def norm_half(jp, Yt):
                    """normalize y[q,0:64] /= y[q,64] for q-blocks 2jp,2jp+1
                    into yt (DVE only; shipping happens once per q-tile)."""
                    recip = rpool.tile([128, 4], f32, name="recip")
                    nc.vector.reciprocal(
                        recip[:].rearrange("p (j c) -> p j c", c=1),
                        Yt.rearrange("p (j c) -> p j c", c=65)[:, :, 64:65])
                    for jj in (0, 1):
                        j = 2 * jp + jj
                        for h01 in (0, 1):
                            nc.vector.tensor_scalar_mul(
                                yt[:, j * 128 + h01 * 64: j * 128 + (h01 + 1) * 64],
                                Yt[:, jj * 130 + h01 * 65: jj * 130 + h01 * 65 + 64],
                                recip[:, jj * 2 + h01: jj * 2 + h01 + 1])

                for u in (1, 0):
                    bd = qkps.tile([128, 1024], f32, name="qk", tag="qk")
                    for i in (0, 1):
                        for h01 in (0, 1):
                            qk_mm(bd[:, (h01 * 2 + i) * 256:(h01 * 2 + i + 1) * 256],
                                  g + 2 * u + i, q0 + u * 256, 256, h01)
                    eb = epool.tile([128, 1024], bf16, name="e")
                    nc.scalar.activation(eb[:], bd[:],
                                         mybir.ActivationFunctionType.Exp,
                                         scale=1.0 / np.sqrt(HD))
                    act_tile()
                    nc.vector.tensor_mul(eb[:], eb[:], mask_sb[:])
                    avs = []
                    for h01 in (0, 1):
                        for i in (0, 1):
                            for jj in (0, 1):
                                if i == 1 and jj == 0:
                                    continue    # fully masked-out slot
                                avs.append(
                                    (lambda e_=eb, h_=h01, i_=i, jj_=jj, u_=u:
                                     av_mm(e_, (h_ * 2 + i_) * 256 + jj_ * 128,
                                           2 * u_ + jj_, h_, g + 2 * u_ + i_,
                                           (f'b{u_}', i_))))
                    stage_tile(eb, avs)
                    if u == 0:
                        # band1's AVs were just flushed -> Ycd is closed
                        norm_half(1, Ycd)
                flush_pend()
                norm_half(0, Yab)
                # one transpose + one agin per q-tile: keeps the sync HWDGE
                # queues sparse so FIFO sem thresholds resolve promptly
                in_t, out_t, coff, fire, qis = ship_plan[(pair, qi)]
                nc.sync.dma_start_transpose(
                    ytT[:].rearrange("p (j q) -> p j q", j=4), yt[:])
                nc.sync.dma_start(in_t[:, coff:coff + QT], ytT[:])
                if (pair, qi) == (1, 2):
                    # queue-padding: shift the next q-tile's ship DMAs to
                    # different HWDGE queue slots so the (1,2) collective's
                    # conservative queue-sem threshold lands on these
                    # instantly-completing dummies instead of qi3's ships
                    for _ in range(4):
                        nc.sync.dma_start(dummy_sb[:], dummy_dr[:])
                if fire:
                    gl = in_t.shape[1]
                    nc.gpsimd.collective_compute(
                        "AllGather", mybir.AluOpType.bypass,
                        replica_groups=GROUPS,
                        ins=[in_t.opt()], outs=[out_t.opt()])
                    done_t = max(est[0], est[1]) + 11000.0
                    # chained pulls: emit the PREVIOUS collective's pull now,
                    # just after dispatching this one. A pull parks Pool.SEQ
                    # until its collective completes; chaining keeps that park
                    # from delaying a collective dispatch, and keeps parked
                    # DMAs off the sync HWDGE queues (whose FIFO semaphores
                    # would delay every later DMA sharing the queue).
                    t = ygp.tile([128, 4 * gl], bf16, name="yg",
                                 tag=f"yg{gl}", bufs=(6 if gl == QT else 1))
                    for pq in qis:
                        coll_done[pq] = done_t
                        ygs[pq] = (t, gl, (pq[1] - qis[0][1]) * QT)
                    flush_pull()
                    pend_pull[0] = (t, out_t, gl)

            # ================= main schedule =================
            def make_pre_diag(tag_k, tag_v, qi):
                def f():
                    force((tag_k, qi))
                    for c in range(4 * qi, 4 * qi + 4):
                        force((tag_v, c))
                return f

            for qi in range(nqt):
                force(('q0', qi))
                for n in range(qi):
                    force(('k0', n))
                for c in range(4 * qi):
                    force(('v0', c))
                attention_qi(0, qi, pre_diag=make_pre_diag('k0', 'v0', qi))

            for qi in range(nqt):
                force(('q1', qi))
                for n in range(qi):
                    force(('k1', n))
                for c in range(4 * qi):
                    force(('v1', c))
                extra = []
                if qi == 3:
                    extra += [(0.05, 'unit', ('op', 0)),
                              (0.35, 'unit', ('op', 1)),
                              (0.85, 'unit', ('op', 2))]
                attention_qi(1, qi, extra,
                             pre_diag=make_pre_diag('k1', 'v1', qi))
                for tq in ([0, 1] if qi == 1 else [qi] if qi >= 2 else []):
                    add_unit(('op', tq),
                             max(coll_done[(0, tq)], coll_done[(1, tq)],
                                 wo_ready) + 1000.0,
                             o_proj_gen(tq), QT)

            flush_pull()
            for tq in range(nqt):
                force(('op', tq))
            # flush any remaining deferred events (out DMAs of the tail)
            for ev in events:
                if not ev[2]:
                    ev[1]()
                    ev[2] = True

    nc.compile()
    return nc


def _masks_np():
    """Diagonal causal mask: [ki, qi] = qi >= ki, duplicated along the free
    axis for the two packed heads."""
    ki = np.arange(128)[:, None]
    qi = np.arange(128)[None, :]
    tri = (qi >= ki).astype(np.float32)
    ones = np.ones((128, 128), np.float32)
    zeros = np.zeros((128, 128), np.float32)
    lo = np.concatenate([tri, ones], axis=1)    # lower k-block of a band
    hi = np.concatenate([zeros, tri], axis=1)   # upper k-block of a band
    return np.concatenate([lo, hi, lo, hi], axis=1).astype(BF16)  # [128, 1024]


def _block(a, w):
    """[C, w] -> [128, NCB*w] partition-blocked bf16."""
    return np.ascontiguousarray(
        a.reshape(NCB, 128, w).transpose(1, 0, 2).reshape(128, NCB * w)).astype(BF16)


def _prep_inputs(x, Wq, Wk, Wv, Wo, t_len):
    masks = _masks_np()
    in_maps = []
    for c in range(N_CORES):
        b, hg = divmod(c, 4)
        sl = slice(hg * SH, (hg + 1) * SH)
        in_maps.append({
            "xT": _block(x[b].T, t_len),
            "wqT": _block(Wq[sl, :].T, SH),
            "wkT": _block(Wk[sl, :].T, SH),
            "wvT": _block(Wv[sl, :].T, SH),
            "woT": _block(Wo[sl, :].T, SH),
            "masks": masks,
        })
    return in_maps


def _assemble(results, t_len):
    out = np.empty((B, t_len, C), dtype=np.float32)
    for c in range(N_CORES):
        b, hg = divmod(c, 4)
        out[b, :, hg * SH:(hg + 1) * SH] = results[c]["out"].T.astype(np.float32)
    return out


def get_nc(t_len=T):
    if t_len not in _CACHE:
        _CACHE[t_len] = _build(t_len)
    return _CACHE[t_len]


def kernel(x, Wq, Wk, Wv, Wo):
    from concourse import bass_utils
    x = np.asarray(x, dtype=np.float32)
    nc = get_nc(T)
    in_maps = _prep_inputs(x, np.asarray(Wq), np.asarray(Wk), np.asarray(Wv),
                           np.asarray(Wo), T)
    res = bass_utils.run_bass_kernel_spmd(nc, in_maps, core_ids=list(range(N_CORES)))
    return _assemble(res.results, T)


# revision 79
# speedup vs baseline: 1.0014x; 1.0014x over previous
"""Distributed causal self-attention kernel for one TRN2 chip (8 NeuronCores).

Problem: y = CausalSelfAttention(x) with B=2, T=2048, C=1024, 16 heads x 64.

Sharding (per core c = b*4 + hg;  b = batch, hg = head-group of 4 heads):
  - Q/K/V projections: column-sharded per head group (each core computes its
    4 heads' Q,K,V from the full x of its batch).
  - Attention: fully local (4 heads per core), flash-style. Scores are kept
    transposed (s^T[k, q]); the AV matmul is emitted in the *output-
    transposed* orientation (out y[q, d], lhsT = e-slice, rhs = V) so each
    AV streams only 65 columns instead of 512 -- half the PE time.
  - Row-sums for softmax ride the AV matmul as a 65th "ones" column of V.
  - y[q, d] is normalized on DVE, transposed back to y^T[d, q] with the DMA
    xbar (dma_start_transpose), and AllGathered per (pair, 512-q-tile):
    8 small collectives spread across the timeline instead of 4 big ones.
  - o_proj: each core computes its own 256 output columns from the gathered
    y^T -> output shards are disjoint; the host just concatenates.

Scheduling: PE executes in issue order, and the attention inner loop is
Activation-bound (exp on [128,1024] tiles ~1us vs ~0.65us of PE work per
k-block). The emitter therefore software-pipelines exp one k-block ahead
and weaves "filler" matmul units (the other pair's projections, o_proj
tiles) into the attention stream, guided by an estimated-time clock that
knows DMA arrival and collective completion times. Input DMA is ordered by
first use (wq, x t-tile 0, wk, masks, ...) so attention starts ~3us in.

All matmuls run in bf16 (fp32 accumulation in PSUM); inputs are converted
to bf16 on the host. QK^T matmuls (contraction dim 64) are packed
two-per-PE via tile_position row tiling.
"""
import sys
sys.path.insert(0, '/opt/trn_rl_repo')
import numpy as np
import ml_dtypes

B, T, C = 2, 2048, 1024
NH, HD = 16, 64
N_CORES = 8
GROUPS = [[0, 1, 2, 3], [4, 5, 6, 7]]
HPC = NH // 4            # heads per core = 4
SH = HPC * HD            # per-core projection width = 256
NCB = C // 128           # contraction blocks = 8
QT = 512                 # query tile
VW = HPC * 65            # vhat row width = 260
BF16 = ml_dtypes.bfloat16

PE_NS = 1.0 / 2.4        # ns per PE column-cycle at full clock
EXP_NS = 1000.0          # Act time for one [128,1024] exp tile
DMA_BPS = 320e9          # effective DMA_ENGINES bandwidth for estimates

_CACHE = {}


def _build(t_len):
    import concourse.bass as bass
    import concourse.bacc as bacc
    import concourse.tile as tile
    import concourse.mybir as mybir
    dt = mybir.dt
    f32, bf16 = dt.float32, dt.bfloat16

    nqt = t_len // QT        # query tiles per pair = 4
    ntc = t_len // 128       # t chunks of 128 = 16

    nc = bacc.Bacc("TRN2", target_bir_lowering=False, debug=False,
                   num_devices=N_CORES)
    # inputs arrive pre-blocked on the host: [(cblk p) ...] -> [p, cblk*...]
    xT = nc.dram_tensor("xT", [128, NCB * t_len], bf16, kind="ExternalInput")
    wq = nc.dram_tensor("wqT", [128, NCB * SH], bf16, kind="ExternalInput")
    wk = nc.dram_tensor("wkT", [128, NCB * SH], bf16, kind="ExternalInput")
    wv = nc.dram_tensor("wvT", [128, NCB * SH], bf16, kind="ExternalInput")
    wo = nc.dram_tensor("woT", [128, NCB * SH], bf16, kind="ExternalInput")
    masks = nc.dram_tensor("masks", [128, 1024], bf16, kind="ExternalInput")
    out = nc.dram_tensor("out", [SH, t_len], bf16, kind="ExternalOutput")

    with tile.TileContext(nc) as tc:
        with tc.tile_pool(name="big", bufs=1) as big, \
             tc.tile_pool(name="epool", bufs=4) as epool, \
             tc.tile_pool(name="ytp", bufs=3) as ytp, \
             tc.tile_pool(name="yttp", bufs=3) as yttp, \
             tc.tile_pool(name="rpool", bufs=4) as rpool, \
             tc.tile_pool(name="ygp", bufs=8) as ygp, \
             tc.tile_pool(name="stp", bufs=3) as stp, \
             tc.tile_pool(name="qkps", bufs=2, space="PSUM") as qkps, \
             tc.tile_pool(name="yps", bufs=1, space="PSUM") as yps, \
             tc.tile_pool(name="fps", bufs=2, space="PSUM") as fps, \
             tc.tile_pool(name="dram", bufs=1, space="DRAM") as dram:

            # ---- resident SBUF tensors ----
            xt = big.tile([128, NCB * t_len], bf16)       # x^T, c-blocked
            wq_sb = big.tile([128, NCB * SH], bf16)
            wk_sb = big.tile([128, NCB * SH], bf16)
            wv_sb = big.tile([128, NCB * SH], bf16)
            wo_sb = big.tile([128, NCB * SH], bf16)
            mask_sb = big.tile([128, 1024], bf16)
            qt_sb = big.tile([128, 2 * t_len], bf16)      # Q^T, pair-blocked
            kt_sb = big.tile([128, 2 * t_len], bf16)
            vhat_sb = big.tile([128, ntc * VW], bf16)     # [V_h | 1] per head

            # ones column of vhat (col 64 of each head's 65-wide slot)
            nc.gpsimd.memset(
                vhat_sb.rearrange("p (t h c) -> p t h c", h=HPC, c=65)[:, :, :, 64:65],
                1.0)

            # ---- input DMA: few large transfers, ordered by first consumer ----
            # Estimated arrival clock for the scheduler.
            dma_cum = [0.0]

            def stage(dst, src, nbytes):
                nc.sync.dma_start(dst, src)
                dma_cum[0] += nbytes
                return 2000.0 + dma_cum[0] / DMA_BPS * 1e9

            def stage_x(tq, kblk=None):
                kb = slice(None) if kblk is None else slice(*kblk)
                nk = NCB if kblk is None else kblk[1] - kblk[0]
                sl = (slice(None), kb, slice(tq * QT, (tq + 1) * QT))
                return stage(
                    xt.rearrange("p (k t) -> p k t", k=NCB)[sl],
                    xT.rearrange("p (k t) -> p k t", k=NCB)[sl],
                    128 * nk * QT * 2)

            def stage_wh(w_sb, w_in, pair, k0=0):
                """one pair's 128-column half of a weight tensor, strided.
                256B elements pay the sub-512B descriptor penalty -> bill 2x."""
                sl = (slice(None), slice(k0, None), slice(pair * 128, (pair + 1) * 128))
                return stage(
                    w_sb.rearrange("p (k c) -> p k c", k=NCB)[sl],
                    w_in.rearrange("p (k c) -> p k c", k=NCB)[sl],
                    128 * (NCB - k0) * 128 * 2 * 2)

            WB = 128 * NCB * SH * 2    # bytes of one full weight tensor
            x_ready = [0.0] * nqt
            # tiny first chunks so the very first matmul starts ~2us earlier
            stage(wq_sb[:, 0:128], wq[:, 0:128], 128 * 128 * 2)
            stage(xt[:, 0:QT], xT[:, 0:QT], 128 * QT * 2)
            wq_ready = stage_wh(wq_sb, wq, 0, k0=1)
            stage_x(0, (1, 4))
            x_ready[0] = stage_x(0, (4, 8))
            wk_ready = stage_wh(wk_sb, wk, 0)
            mask_ready = stage(mask_sb[:], masks[:], 128 * 1024 * 2)
            wv_ready = stage_wh(wv_sb, wv, 0)
            x_ready[1] = stage_x(1)
            wq1_ready = stage_wh(wq_sb, wq, 1)
            wk1_ready = stage_wh(wk_sb, wk, 1)
            x_ready[2] = stage_x(2)
            wv1_ready = stage_wh(wv_sb, wv, 1)
            x_ready[3] = stage_x(3)
            wo_ready = stage(wo_sb[:], wo[:], WB)
            wk0_ready = wk_ready
            wv0_ready = wv_ready

            # ---- DRAM bounce buffers ----
            # pair 0: one AllGather per q-tile (fine overlap with attn(0)).
            # pair 1: q-tiles 0+1 share one gather -- less COLLECTIVE_CORES
            # backlog in the contended end-of-kernel window.
            # ship_plan[(pair, qi)] = (in_tile, out_tile, col_off, fire, qis)
            ship_plan = {}
            for q in range(nqt):
                i_t = dram.tile([128, QT], bf16, name=f"agin0{q}")
                o_t = dram.tile([512, QT], bf16, name=f"agout0{q}")
                ship_plan[(0, q)] = (i_t, o_t, 0, True, [(0, q)])
            ag1a_i = dram.tile([128, 2 * QT], bf16, name="agin1a")
            ag1a_o = dram.tile([512, 2 * QT], bf16, name="agout1a")
            ship_plan[(1, 0)] = (ag1a_i, ag1a_o, 0, False, [])
            ship_plan[(1, 1)] = (ag1a_i, ag1a_o, QT, True, [(1, 0), (1, 1)])
            for q in (2, 3):
                i_t = dram.tile([128, QT], bf16, name=f"agin1{q}")
                o_t = dram.tile([512, QT], bf16, name=f"agout1{q}")
                ship_plan[(1, q)] = (i_t, o_t, 0, True, [(1, q)])

            # ================= scheduler machinery =================
            # est[0] = PE-work cursor (ns), est[1] = Act cursor (ns).
            est = [2000.0, 0.0]

            def pe(cols):
                est[0] += cols * PE_NS

            def act_tile():
                est[1] = max(est[1], est[0]) + EXP_NS

            class Unit:
                """A filler work unit: generator emitting matmuls lazily."""

                def __init__(self, key, ready, gen, cols_per_step):
                    self.key, self.ready, self.gen = key, ready, gen
                    self.cols = cols_per_step
                    self.done = False

                def step(self):
                    try:
                        next(self.gen)
                        pe(self.cols)
                        return True
                    except StopIteration:
                        self.done = True
                        return False

            units = []          # ordered list of Units
            by_key = {}

            def add_unit(key, ready, gen, cols):
                u = Unit(key, ready, gen, cols)
                units.append(u)
                by_key[key] = u

            # deferred emission events (e.g. agout pulls): emitted once the
            # est clock passes `ready`, so their sem waits resolve quickly
            # and never park long on an engine SEQ.
            events = []

            def at_time(ready, fn, key=None):
                events.append([ready, fn, False, key])

            def pump():
                now = max(est[0], est[1])
                for ev in events:
                    if not ev[2] and now >= ev[0]:
                        ev[1]()
                        ev[2] = True

            def pump_force(key):
                for ev in events:
                    if not ev[2] and ev[3] == key:
                        ev[1]()
                        ev[2] = True

            active = [None]

            def _next_active():
                if active[0] is not None and not active[0].done:
                    return active[0]
                now = max(est[0], est[1])
                for u in units:
                    if not u.done and u.ready <= now:
                        active[0] = u
                        return u
                return None

            def fill(budget_ns):
                while budget_ns > 0:
                    u = _next_active()
                    if u is None or not u.step():
                        if u is None:
                            return
                        continue
                    budget_ns -= u.cols * PE_NS

            def force(key):
                u = by_key[key]
                if u.done:
                    return
                est[0] = max(est[0], u.ready)
                while u.step():
                    pass

            # ================= work-unit generators =================
            def qk_proj_gen(pair, w_sb, dst_sb, n):
                """Q^T/K^T projection tile n for one pair: 8 matmuls + copy."""
                ps = fps.tile([128, QT], f32, name="fp", tag="fp")
                for k in range(NCB):
                    nc.tensor.matmul(
                        ps[:],
                        lhsT=w_sb[:, k * SH + pair * 128: k * SH + (pair + 1) * 128],
                        rhs=xt[:, k * t_len + n * QT: k * t_len + (n + 1) * QT],
                        start=(k == 0), stop=(k == NCB - 1))
                    yield
                nc.vector.tensor_copy(
                    dst_sb[:, pair * t_len + n * QT: pair * t_len + (n + 1) * QT],
                    ps[:])

            def v_proj_gen(pair, tch):
                """V (2 heads) for t-chunk tch, written into vhat slots."""
                ps = fps.tile([128, 128], f32, name="fp", tag="fp")
                for k in range(NCB):
                    nc.tensor.matmul(
                        ps[:],
                        lhsT=xt[:, k * t_len + tch * 128: k * t_len + (tch + 1) * 128],
                        rhs=wv_sb[:, k * SH + pair * 128: k * SH + (pair + 1) * 128],
                        start=(k == 0), stop=(k == NCB - 1))
                    yield
                dst = vhat_sb.rearrange("p (t h c) -> p t h c", h=HPC, c=65)[
                    :, tch, 2 * pair: 2 * pair + 2, 0:64]
                nc.vector.tensor_copy(
                    dst, ps.rearrange("p (h c) -> p h c", h=2))

            ygs = {}            # (pair, qi) -> (pulled tile, group_len, off)
            pend_pull = [None]  # (sbuf tile, dram out tile, group_len)

            def flush_pull():
                if pend_pull[0] is not None:
                    t_, o_, gl_ = pend_pull[0]
                    # split merged-group pulls per 512-q-tile: the first
                    # o_proj tile only needs the first half, ~1.5us sooner
                    nsp = gl_ // QT
                    for sp in range(nsp):
                        nc.gpsimd.dma_start(
                            t_[:].rearrange("p (r s t) -> p r s t",
                                            r=4, s=nsp)[:, :, sp],
                            o_.rearrange("(r p) (s t) -> p r s t",
                                         r=4, s=nsp)[:, :, sp],
                        )
                    pend_pull[0] = None

            def o_proj_gen(tq):
                """o_proj for t-tile tq: 16 matmuls from pulled y^T, store."""
                def yg(cb):
                    r, p2 = divmod(cb, 2)
                    t_, gl_, off_ = ygs[(p2, tq)]
                    return t_[:, r * gl_ + off_: r * gl_ + off_ + QT]
                st = stp.tile([128, 2 * QT], bf16, name="st")
                for m in range(2):
                    ps = fps.tile([128, QT], f32, name="fp", tag="fp")
                    for cb in range(NCB):
                        nc.tensor.matmul(
                            ps[:],
                            lhsT=wo_sb[:, cb * SH + m * 128: cb * SH + (m + 1) * 128],
                            rhs=yg(cb),
                            start=(cb == 0), stop=(cb == NCB - 1))
                        yield
                    nc.vector.tensor_copy(st[:, m * QT:(m + 1) * QT], ps[:])
                    # per-m store: m0 ships while m1 still accumulates
                    nc.sync.dma_start(
                        out[m * 128:(m + 1) * 128, tq * QT:(tq + 1) * QT],
                        st[:, m * QT:(m + 1) * QT])

            # register projection units
            for n in range(nqt):
                add_unit(('q0', n), x_ready[n],
                         qk_proj_gen(0, wq_sb, qt_sb, n), QT)
                add_unit(('k0', n), max(x_ready[n], wk0_ready),
                         qk_proj_gen(0, wk_sb, kt_sb, n), QT)
            for c in range(ntc):
                add_unit(('v0', c), max(x_ready[c // 4], wv0_ready),
                         v_proj_gen(0, c), 128)
            for n in range(nqt):
                add_unit(('q1', n), max(x_ready[n], wq1_ready),
                         qk_proj_gen(1, wq_sb, qt_sb, n), QT)
                add_unit(('k1', n), max(x_ready[n], wk1_ready),
                         qk_proj_gen(1, wk_sb, kt_sb, n), QT)
            for c in range(ntc):
                add_unit(('v1', c), max(x_ready[c // 4], wv1_ready),
                         v_proj_gen(1, c), 128)

            coll_done = {}      # (pair, qi) -> est completion ns

            # ================= attention =================
            def attention_qi(pair, qi, extra=(), pre_diag=None):
                """extra: list of (frac, kind, key) actions fired when the
                emission reaches `frac` of this qi's exp tiles. kind 'pump'
                force-emits a deferred event; kind 'unit' unlocks a filler
                unit (pumping its pulls first)."""
                g = 4 * qi
                q0 = qi * QT
                total_tiles = g + 3
                tcount = [0]
                pend_extra = list(extra)

                def poll_extra():
                    frac = tcount[0] / total_tiles
                    for ex in list(pend_extra):
                        if frac >= ex[0]:
                            if ex[1] == 'pump':
                                pump_force(ex[2])
                            else:
                                flush_pull()
                                if ex[2] in by_key:
                                    by_key[ex[2]].ready = 0.0
                            pend_extra.remove(ex)

                def qk_mm(dst, kb, qa, w, h01):
                    nc.tensor.matmul(
                        dst,
                        lhsT=kt_sb[h01 * 64:(h01 + 1) * 64,
                                   pair * t_len + kb * 128: pair * t_len + (kb + 1) * 128],
                        rhs=qt_sb[h01 * 64:(h01 + 1) * 64,
                                  pair * t_len + qa: pair * t_len + qa + w],
                        start=True, stop=True,
                        tile_position=(h01 * 64, 0))
                    pe(w)

                Yab = yps.tile([128, 260], f32, name="Yab", tag="Yab")
                Ycd = yps.tile([128, 260], f32, name="Ycd", tag="Ycd")
                Y = {0: (Yab, 0), 1: (Yab, 1), 2: (Ycd, 0), 3: (Ycd, 1)}
                started = set()
                stop_at = {0: ('b0', 0), 1: ('b0', 1),
                           2: ('b1', 0), 3: ('b1', 1)}

                def av_mm(e_tile, ecol, j, h01, kb, tag):
                    yt_, jj = Y[j]
                    # ONE start=True per Y tile per round: start marks the
                    # whole PSUM zero-region (bank) pending-zero, so each
                    # slot's first write then overwrites and later writes
                    # accumulate. A second start in the same bank would
                    # re-poison already-written slots.
                    key = id(yt_)
                    st_ = key not in started
                    started.add(key)
                    nc.tensor.matmul(
                        yt_[:, jj * 130 + h01 * 65: jj * 130 + (h01 + 1) * 65],
                        lhsT=e_tile[:, ecol: ecol + 128],
                        rhs=vhat_sb[:, kb * VW + (2 * pair + h01) * 65:
                                    kb * VW + (2 * pair + h01 + 1) * 65],
                        start=st_, stop=(stop_at[j] == tag),
                        skip_group_check=True)
                    pe(65)

                pend = None     # deferred AV list from the previous tile

                def flush_pend():
                    nonlocal pend
                    if pend is not None:
                        est[0] = max(est[0], pend[0])
                        for f in pend[1]:
                            f()
                        pend = None

                def stage_tile(e_tile, avs):
                    nonlocal pend
                    flush_pend()
                    tcount[0] += 1
                    poll_extra()
                    pump()
                    fill(max(0.0, (est[1] - est[0]) - 600.0))
                    pend = (est[1], [])
                    for a in avs:
                        pend[1].append(a)

                # full k-blocks
                for kb in range(g):
                    qk = qkps.tile([128, 1024], f32, name="qk", tag="qk")
                    for h01 in (0, 1):
                        qk_mm(qk[:, h01 * 512:(h01 + 1) * 512], kb, q0, 512, h01)
                    e = epool.tile([128, 1024], bf16, name="e")
                    nc.scalar.activation(e[:], qk[:],
                                         mybir.ActivationFunctionType.Exp,
                                         scale=1.0 / np.sqrt(HD))
                    act_tile()
                    avs = []
                    for h01 in (0, 1):
                        for j in range(4):
                            avs.append(
                                (lambda e_=e, h_=h01, j_=j, kb_=kb:
                                 av_mm(e_, h_ * 512 + j_ * 128, j_, h_, kb_,
                                       ('full', kb_))))
                    stage_tile(e, avs)

                # K tile qi / V chunks 4qi..4qi+3 are first needed here; a
                # late force keeps the head of the q-tile exp-dense
                if pre_diag is not None:
                    pre_diag()

                # diagonal: mid supertile (kb g,g+1 vs upper q-half, unmasked)
                mid = qkps.tile([128, 1024], f32, name="qk", tag="qk")
                for i in (0, 1):
                    for h01 in (0, 1):
                        qk_mm(mid[:, (h01 * 2 + i) * 256:(h01 * 2 + i + 1) * 256],
                              g + i, q0 + 256, 256, h01)
                em = epool.tile([128, 1024], bf16, name="e")
                nc.scalar.activation(em[:], mid[:],
                                     mybir.ActivationFunctionType.Exp,
                                     scale=1.0 / np.sqrt(HD))
                act_tile()
                avs = []
                for h01 in (0, 1):
                    for i in (0, 1):
                        for jj in (0, 1):   # j = 2 + jj
                            avs.append(
                                (lambda e_=em, h_=h01, i_=i, jj_=jj:
                                 av_mm(e_, (h_ * 2 + i_) * 256 + jj_ * 128,
                                       2 + jj_, h_, g + i_, ('mid', i_))))
                stage_tile(em, avs)

                # diagonal bands (masked): band u covers q-half u vs kb g+2u+{0,1}
                # band order (1, 0): Ycd (q-blocks 2,3) closes one tile early
                # so its normalize/ship overlaps band0's compute.
                yt = ytp.tile([128, QT], bf16, name="yt")
                ytT = yttp.tile([128, QT], bf16, name="ytT")

                def norm_half(jp, Yt):
                    """normalize y[q,0:64] /= y[q,64] for q-blocks 2jp,2jp+1
                    into yt (DVE only; shipping happens once per q-tile)."""
                    recip = rpool.tile([128, 4], f32, name="recip")
                    nc.vector.reciprocal(
                        recip[:].rearrange("p (j c) -> p j c", c=1),
                        Yt.rearrange("p (j c) -> p j c", c=65)[:, :, 64:65])
                    for jj in (0, 1):
                        j = 2 * jp + jj
                        for h01 in (0, 1):
                            nc.vector.tensor_scalar_mul(
                                yt[:, j * 128 + h01 * 64: j * 128 + (h01 + 1) * 64],
                                Yt[:, jj * 130 + h01 * 65: jj * 130 + h01 * 65 + 64],
                                recip[:, jj * 2 + h01: jj * 2 + h01 + 1])

                for u in (1, 0):
                    bd = qkps.tile([128, 1024], f32, name="qk", tag="qk")
                    for i in (0, 1):
                        for h01 in (0, 1):
                            qk_mm(bd[:, (h01 * 2 + i) * 256:(h01 * 2 + i + 1) * 256],
                                  g + 2 * u + i, q0 + u * 256, 256, h01)
                    eb = epool.tile([128, 1024], bf16, name="e")
                    nc.scalar.activation(eb[:], bd[:],
                                         mybir.ActivationFunctionType.Exp,
                                         scale=1.0 / np.sqrt(HD))
                    act_tile()
                    nc.vector.tensor_mul(eb[:], eb[:], mask_sb[:])
                    avs = []
                    for h01 in (0, 1):
                        for i in (0, 1):
                            for jj in (0, 1):
                                if i == 1 and jj == 0:
                                    continue    # fully masked-out slot
                                avs.append(
                                    (lambda e_=eb, h_=h01, i_=i, jj_=jj, u_=u:
                                     av_mm(e_, (h_ * 2 + i_) * 256 + jj_ * 128,
                                           2 * u_ + jj_, h_, g + 2 * u_ + i_,
                                           (f'b{u_}', i_))))
                    stage_tile(eb, avs)
                    if u == 0:
                        # band1's AVs were just flushed -> Ycd is closed
                        norm_half(1, Ycd)
                flush_pend()
                norm_half(0, Yab)
                # one transpose + one agin per q-tile: keeps the sync HWDGE
                # queues sparse so FIFO sem thresholds resolve promptly
                in_t, out_t, coff, fire, qis = ship_plan[(pair, qi)]
                nc.sync.dma_start_transpose(
                    ytT[:].rearrange("p (j q) -> p j q", j=4), yt[:])
                nc.sync.dma_start(in_t[:, coff:coff + QT], ytT[:])
                if fire:
                    gl = in_t.shape[1]
                    nc.gpsimd.collective_compute(
                        "AllGather", mybir.AluOpType.bypass,
                        replica_groups=GROUPS,
                        ins=[in_t.opt()], outs=[out_t.opt()])
                    done_t = max(est[0], est[1]) + 11000.0
                    # chained pulls: emit the PREVIOUS collective's pull now,
                    # just after dispatching this one. A pull parks Pool.SEQ
                    # until its collective completes; chaining keeps that park
                    # from delaying a collective dispatch, and keeps parked
                    # DMAs off the sync HWDGE queues (whose FIFO semaphores
                    # would delay every later DMA sharing the queue).
                    t = ygp.tile([128, 4 * gl], bf16, name="yg",
                                 tag=f"yg{gl}", bufs=(6 if gl == QT else 1))
                    for pq in qis:
                        coll_done[pq] = done_t
                        ygs[pq] = (t, gl, (pq[1] - qis[0][1]) * QT)
                    flush_pull()
                    pend_pull[0] = (t, out_t, gl)

            # ================= main schedule =================
            def make_pre_diag(tag_k, tag_v, qi):
                def f():
                    force((tag_k, qi))
                    for c in range(4 * qi, 4 * qi + 4):
                        force((tag_v, c))
                return f

            for qi in range(nqt):
                force(('q0', qi))
                for n in range(qi):
                    force(('k0', n))
                for c in range(4 * qi):
                    force(('v0', c))
                attention_qi(0, qi, pre_diag=make_pre_diag('k0', 'v0', qi))

            for qi in range(nqt):
                force(('q1', qi))
                for n in range(qi):
                    force(('k1', n))
                for c in range(4 * qi):
                    force(('v1', c))
                extra = []
                if qi == 3:
                    extra += [(0.05, 'unit', ('op', 0)),
                              (0.35, 'unit', ('op', 1)),
                              (0.85, 'unit', ('op', 2))]
                attention_qi(1, qi, extra,
                             pre_diag=make_pre_diag('k1', 'v1', qi))
                for tq in ([0, 1] if qi == 1 else [qi] if qi >= 2 else []):
                    add_unit(('op', tq),
                             max(coll_done[(0, tq)], coll_done[(1, tq)],
                                 wo_ready) + 1000.0,
                             o_proj_gen(tq), QT)

            flush_pull()
            for tq in range(nqt):
                force(('op', tq))
            # flush any remaining deferred events (out DMAs of the tail)
            for ev in events:
                if not ev[2]:
                    ev[1]()
                    ev[2] = True

    nc.compile()
    return nc


def _masks_np():
    """Diagonal causal mask: [ki, qi] = qi >= ki, duplicated along the free
    axis for the two packed heads."""
    ki = np.arange(128)[:, None]
    qi = np.arange(128)[None, :]
    tri = (qi >= ki).astype(np.float32)
    ones = np.ones((128, 128), np.float32)
    zeros = np.zeros((128, 128), np.float32)
    lo = np.concatenate([tri, ones], axis=1)    # lower k-block of a band
    hi = np.concatenate([zeros, tri], axis=1)   # upper k-block of a band
    return np.concatenate([lo, hi, lo, hi], axis=1).astype(BF16)  # [128, 1024]


def _block(a, w):
    """[C, w] -> [128, NCB*w] partition-blocked bf16."""
    return np.ascontiguousarray(
        a.reshape(NCB, 128, w).transpose(1, 0, 2).reshape(128, NCB * w)).astype(BF16)


def _prep_inputs(x, Wq, Wk, Wv, Wo, t_len):
    masks = _masks_np()
    in_maps = []
    for c in range(N_CORES):
        b, hg = divmod(c, 4)
        sl = slice(hg * SH, (hg + 1) * SH)
        in_maps.append({
            "xT": _block(x[b].T, t_len),
            "wqT": _block(Wq[sl, :].T, SH),
            "wkT": _block(Wk[sl, :].T, SH),
            "wvT": _block(Wv[sl, :].T, SH),
            "woT": _block(Wo[sl, :].T, SH),
            "masks": masks,
        })
    return in_maps


def _assemble(results, t_len):
    out = np.empty((B, t_len, C), dtype=np.float32)
    for c in range(N_CORES):
        b, hg = divmod(c, 4)
        out[b, :, hg * SH:(hg + 1) * SH] = results[c]["out"].T.astype(np.float32)
    return out


def get_nc(t_len=T):
    if t_len not in _CACHE:
        _CACHE[t_len] = _build(t_len)
    return _CACHE[t_len]


def kernel(x, Wq, Wk, Wv, Wo):
    from concourse import bass_utils
    x = np.asarray(x, dtype=np.float32)
    nc = get_nc(T)
    in_maps = _prep_inputs(x, np.asarray(Wq), np.asarray(Wk), np.asarray(Wv),
                           np.asarray(Wo), T)
    res = bass_utils.run_bass_kernel_spmd(nc, in_maps, core_ids=list(range(N_CORES)))
    return _assemble(res.results, T)


# revision 80
# speedup vs baseline: 1.0023x; 1.0008x over previous
"""Distributed causal self-attention kernel for one TRN2 chip (8 NeuronCores).

Problem: y = CausalSelfAttention(x) with B=2, T=2048, C=1024, 16 heads x 64.

Sharding (per core c = b*4 + hg;  b = batch, hg = head-group of 4 heads):
  - Q/K/V projections: column-sharded per head group (each core computes its
    4 heads' Q,K,V from the full x of its batch).
  - Attention: fully local (4 heads per core), flash-style. Scores are kept
    transposed (s^T[k, q]); the AV matmul is emitted in the *output-
    transposed* orientation (out y[q, d], lhsT = e-slice, rhs = V) so each
    AV streams only 65 columns instead of 512 -- half the PE time.
  - Row-sums for softmax ride the AV matmul as a 65th "ones" column of V.
  - y[q, d] is normalized on DVE, transposed back to y^T[d, q] with the DMA
    xbar (dma_start_transpose), and AllGathered per (pair, 512-q-tile):
    8 small collectives spread across the timeline instead of 4 big ones.
  - o_proj: each core computes its own 256 output columns from the gathered
    y^T -> output shards are disjoint; the host just concatenates.

Scheduling: PE executes in issue order, and the attention inner loop is
Activation-bound (exp on [128,1024] tiles ~1us vs ~0.65us of PE work per
k-block). The emitter therefore software-pipelines exp one k-block ahead
and weaves "filler" matmul units (the other pair's projections, o_proj
tiles) into the attention stream, guided by an estimated-time clock that
knows DMA arrival and collective completion times. Input DMA is ordered by
first use (wq, x t-tile 0, wk, masks, ...) so attention starts ~3us in.

All matmuls run in bf16 (fp32 accumulation in PSUM); inputs are converted
to bf16 on the host. QK^T matmuls (contraction dim 64) are packed
two-per-PE via tile_position row tiling.
"""
import sys
sys.path.insert(0, '/opt/trn_rl_repo')
import numpy as np
import ml_dtypes

B, T, C = 2, 2048, 1024
NH, HD = 16, 64
N_CORES = 8
GROUPS = [[0, 1, 2, 3], [4, 5, 6, 7]]
HPC = NH // 4            # heads per core = 4
SH = HPC * HD            # per-core projection width = 256
NCB = C // 128           # contraction blocks = 8
QT = 512                 # query tile
VW = HPC * 65            # vhat row width = 260
BF16 = ml_dtypes.bfloat16

PE_NS = 1.0 / 2.4        # ns per PE column-cycle at full clock
EXP_NS = 1000.0          # Act time for one [128,1024] exp tile
DMA_BPS = 320e9          # effective DMA_ENGINES bandwidth for estimates

_CACHE = {}


def _build(t_len):
    import concourse.bass as bass
    import concourse.bacc as bacc
    import concourse.tile as tile
    import concourse.mybir as mybir
    dt = mybir.dt
    f32, bf16 = dt.float32, dt.bfloat16

    nqt = t_len // QT        # query tiles per pair = 4
    ntc = t_len // 128       # t chunks of 128 = 16

    nc = bacc.Bacc("TRN2", target_bir_lowering=False, debug=False,
                   num_devices=N_CORES)
    # inputs arrive pre-blocked on the host: [(cblk p) ...] -> [p, cblk*...]
    xT = nc.dram_tensor("xT", [128, NCB * t_len], bf16, kind="ExternalInput")
    wq = nc.dram_tensor("wqT", [128, NCB * SH], bf16, kind="ExternalInput")
    wk = nc.dram_tensor("wkT", [128, NCB * SH], bf16, kind="ExternalInput")
    wv = nc.dram_tensor("wvT", [128, NCB * SH], bf16, kind="ExternalInput")
    wo = nc.dram_tensor("woT", [128, NCB * SH], bf16, kind="ExternalInput")
    masks = nc.dram_tensor("masks", [128, 1024], bf16, kind="ExternalInput")
    out = nc.dram_tensor("out", [SH, t_len], bf16, kind="ExternalOutput")

    with tile.TileContext(nc) as tc:
        with tc.tile_pool(name="big", bufs=1) as big, \
             tc.tile_pool(name="epool", bufs=6) as epool, \
             tc.tile_pool(name="ytp", bufs=4) as ytp, \
             tc.tile_pool(name="yttp", bufs=4) as yttp, \
             tc.tile_pool(name="rpool", bufs=6) as rpool, \
             tc.tile_pool(name="ygp", bufs=8) as ygp, \
             tc.tile_pool(name="stp", bufs=4) as stp, \
             tc.tile_pool(name="qkps", bufs=2, space="PSUM") as qkps, \
             tc.tile_pool(name="yps", bufs=1, space="PSUM") as yps, \
             tc.tile_pool(name="fps", bufs=2, space="PSUM") as fps, \
             tc.tile_pool(name="dram", bufs=1, space="DRAM") as dram:

            # ---- resident SBUF tensors ----
            xt = big.tile([128, NCB * t_len], bf16)       # x^T, c-blocked
            wq_sb = big.tile([128, NCB * SH], bf16)
            wk_sb = big.tile([128, NCB * SH], bf16)
            wv_sb = big.tile([128, NCB * SH], bf16)
            wo_sb = big.tile([128, NCB * SH], bf16)
            mask_sb = big.tile([128, 1024], bf16)
            qt_sb = big.tile([128, 2 * t_len], bf16)      # Q^T, pair-blocked
            kt_sb = big.tile([128, 2 * t_len], bf16)
            vhat_sb = big.tile([128, ntc * VW], bf16)     # [V_h | 1] per head

            # ones column of vhat (col 64 of each head's 65-wide slot)
            nc.gpsimd.memset(
                vhat_sb.rearrange("p (t h c) -> p t h c", h=HPC, c=65)[:, :, :, 64:65],
                1.0)

            # ---- input DMA: few large transfers, ordered by first consumer ----
            # Estimated arrival clock for the scheduler.
            dma_cum = [0.0]

            def stage(dst, src, nbytes):
                nc.sync.dma_start(dst, src)
                dma_cum[0] += nbytes
                return 2000.0 + dma_cum[0] / DMA_BPS * 1e9

            def stage_x(tq, kblk=None):
                kb = slice(None) if kblk is None else slice(*kblk)
                nk = NCB if kblk is None else kblk[1] - kblk[0]
                sl = (slice(None), kb, slice(tq * QT, (tq + 1) * QT))
                return stage(
                    xt.rearrange("p (k t) -> p k t", k=NCB)[sl],
                    xT.rearrange("p (k t) -> p k t", k=NCB)[sl],
                    128 * nk * QT * 2)

            def stage_wh(w_sb, w_in, pair, k0=0):
                """one pair's 128-column half of a weight tensor, strided.
                256B elements pay the sub-512B descriptor penalty -> bill 2x."""
                sl = (slice(None), slice(k0, None), slice(pair * 128, (pair + 1) * 128))
                return stage(
                    w_sb.rearrange("p (k c) -> p k c", k=NCB)[sl],
                    w_in.rearrange("p (k c) -> p k c", k=NCB)[sl],
                    128 * (NCB - k0) * 128 * 2 * 2)

            WB = 128 * NCB * SH * 2    # bytes of one full weight tensor
            x_ready = [0.0] * nqt
            # tiny first chunks so the very first matmul starts ~2us earlier
            stage(wq_sb[:, 0:128], wq[:, 0:128], 128 * 128 * 2)
            stage(xt[:, 0:QT], xT[:, 0:QT], 128 * QT * 2)
            wq_ready = stage_wh(wq_sb, wq, 0, k0=1)
            stage_x(0, (1, 4))
            x_ready[0] = stage_x(0, (4, 8))
            wk_ready = stage_wh(wk_sb, wk, 0)
            mask_ready = stage(mask_sb[:], masks[:], 128 * 1024 * 2)
            wv_ready = stage_wh(wv_sb, wv, 0)
            x_ready[1] = stage_x(1)
            wq1_ready = stage_wh(wq_sb, wq, 1)
            wk1_ready = stage_wh(wk_sb, wk, 1)
            x_ready[2] = stage_x(2)
            wv1_ready = stage_wh(wv_sb, wv, 1)
            x_ready[3] = stage_x(3)
            wo_ready = stage(wo_sb[:], wo[:], WB)
            wk0_ready = wk_ready
            wv0_ready = wv_ready

            # ---- DRAM bounce buffers ----
            # pair 0: one AllGather per q-tile (fine overlap with attn(0)).
            # pair 1: q-tiles 0+1 share one gather -- less COLLECTIVE_CORES
            # backlog in the contended end-of-kernel window.
            # ship_plan[(pair, qi)] = (in_tile, out_tile, col_off, fire, qis)
            ship_plan = {}
            for q in range(nqt):
                i_t = dram.tile([128, QT], bf16, name=f"agin0{q}")
                o_t = dram.tile([512, QT], bf16, name=f"agout0{q}")
                ship_plan[(0, q)] = (i_t, o_t, 0, True, [(0, q)])
            ag1a_i = dram.tile([128, 2 * QT], bf16, name="agin1a")
            ag1a_o = dram.tile([512, 2 * QT], bf16, name="agout1a")
            ship_plan[(1, 0)] = (ag1a_i, ag1a_o, 0, False, [])
            ship_plan[(1, 1)] = (ag1a_i, ag1a_o, QT, True, [(1, 0), (1, 1)])
            for q in (2, 3):
                i_t = dram.tile([128, QT], bf16, name=f"agin1{q}")
                o_t = dram.tile([512, QT], bf16, name=f"agout1{q}")
                ship_plan[(1, q)] = (i_t, o_t, 0, True, [(1, q)])

            # ================= scheduler machinery =================
            # est[0] = PE-work cursor (ns), est[1] = Act cursor (ns).
            est = [2000.0, 0.0]

            def pe(cols):
                est[0] += cols * PE_NS

            def act_tile():
                est[1] = max(est[1], est[0]) + EXP_NS

            class Unit:
                """A filler work unit: generator emitting matmuls lazily."""

                def __init__(self, key, ready, gen, cols_per_step):
                    self.key, self.ready, self.gen = key, ready, gen
                    self.cols = cols_per_step
                    self.done = False

                def step(self):
                    try:
                        next(self.gen)
                        pe(self.cols)
                        return True
                    except StopIteration:
                        self.done = True
                        return False

            units = []          # ordered list of Units
            by_key = {}

            def add_unit(key, ready, gen, cols):
                u = Unit(key, ready, gen, cols)
                units.append(u)
                by_key[key] = u

            # deferred emission events (e.g. agout pulls): emitted once the
            # est clock passes `ready`, so their sem waits resolve quickly
            # and never park long on an engine SEQ.
            events = []

            def at_time(ready, fn, key=None):
                events.append([ready, fn, False, key])

            def pump():
                now = max(est[0], est[1])
                for ev in events:
                    if not ev[2] and now >= ev[0]:
                        ev[1]()
                        ev[2] = True

            def pump_force(key):
                for ev in events:
                    if not ev[2] and ev[3] == key:
                        ev[1]()
                        ev[2] = True

            active = [None]

            def _next_active():
                if active[0] is not None and not active[0].done:
                    return active[0]
                now = max(est[0], est[1])
                for u in units:
                    if not u.done and u.ready <= now:
                        active[0] = u
                        return u
                return None

            def fill(budget_ns):
                while budget_ns > 0:
                    u = _next_active()
                    if u is None or not u.step():
                        if u is None:
                            return
                        continue
                    budget_ns -= u.cols * PE_NS

            def force(key):
                u = by_key[key]
                if u.done:
                    return
                est[0] = max(est[0], u.ready)
                while u.step():
                    pass

            # ================= work-unit generators =================
            def qk_proj_gen(pair, w_sb, dst_sb, n):
                """Q^T/K^T projection tile n for one pair: 8 matmuls + copy."""
                ps = fps.tile([128, QT], f32, name="fp", tag="fp")
                for k in range(NCB):
                    nc.tensor.matmul(
                        ps[:],
                        lhsT=w_sb[:, k * SH + pair * 128: k * SH + (pair + 1) * 128],
                        rhs=xt[:, k * t_len + n * QT: k * t_len + (n + 1) * QT],
                        start=(k == 0), stop=(k == NCB - 1))
                    yield
                nc.vector.tensor_copy(
                    dst_sb[:, pair * t_len + n * QT: pair * t_len + (n + 1) * QT],
                    ps[:])

            def v_proj_gen(pair, tch):
                """V (2 heads) for t-chunk tch, written into vhat slots."""
                ps = fps.tile([128, 128], f32, name="fp", tag="fp")
                for k in range(NCB):
                    nc.tensor.matmul(
                        ps[:],
                        lhsT=xt[:, k * t_len + tch * 128: k * t_len + (tch + 1) * 128],
                        rhs=wv_sb[:, k * SH + pair * 128: k * SH + (pair + 1) * 128],
                        start=(k == 0), stop=(k == NCB - 1))
                    yield
                dst = vhat_sb.rearrange("p (t h c) -> p t h c", h=HPC, c=65)[
                    :, tch, 2 * pair: 2 * pair + 2, 0:64]
                nc.vector.tensor_copy(
                    dst, ps.rearrange("p (h c) -> p h c", h=2))

            ygs = {}            # (pair, qi) -> (pulled tile, group_len, off)
            pend_pull = [None]  # (sbuf tile, dram out tile, group_len)

            def flush_pull():
                if pend_pull[0] is not None:
                    t_, o_, gl_ = pend_pull[0]
                    # split merged-group pulls per 512-q-tile: the first
                    # o_proj tile only needs the first half, ~1.5us sooner
                    nsp = gl_ // QT
                    for sp in range(nsp):
                        nc.gpsimd.dma_start(
                            t_[:].rearrange("p (r s t) -> p r s t",
                                            r=4, s=nsp)[:, :, sp],
                            o_.rearrange("(r p) (s t) -> p r s t",
                                         r=4, s=nsp)[:, :, sp],
                        )
                    pend_pull[0] = None

            def o_proj_gen(tq):
                """o_proj for t-tile tq: 16 matmuls from pulled y^T, store."""
                def yg(cb):
                    r, p2 = divmod(cb, 2)
                    t_, gl_, off_ = ygs[(p2, tq)]
                    return t_[:, r * gl_ + off_: r * gl_ + off_ + QT]
                st = stp.tile([128, 2 * QT], bf16, name="st")
                for m in range(2):
                    ps = fps.tile([128, QT], f32, name="fp", tag="fp")
                    for cb in range(NCB):
                        nc.tensor.matmul(
                            ps[:],
                            lhsT=wo_sb[:, cb * SH + m * 128: cb * SH + (m + 1) * 128],
                            rhs=yg(cb),
                            start=(cb == 0), stop=(cb == NCB - 1))
                        yield
                    nc.vector.tensor_copy(st[:, m * QT:(m + 1) * QT], ps[:])
                    # per-m store: m0 ships while m1 still accumulates
                    nc.sync.dma_start(
                        out[m * 128:(m + 1) * 128, tq * QT:(tq + 1) * QT],
                        st[:, m * QT:(m + 1) * QT])

            # register projection units
            for n in range(nqt):
                add_unit(('q0', n), x_ready[n],
                         qk_proj_gen(0, wq_sb, qt_sb, n), QT)
                add_unit(('k0', n), max(x_ready[n], wk0_ready),
                         qk_proj_gen(0, wk_sb, kt_sb, n), QT)
            for c in range(ntc):
                add_unit(('v0', c), max(x_ready[c // 4], wv0_ready),
                         v_proj_gen(0, c), 128)
            for n in range(nqt):
                add_unit(('q1', n), max(x_ready[n], wq1_ready),
                         qk_proj_gen(1, wq_sb, qt_sb, n), QT)
                add_unit(('k1', n), max(x_ready[n], wk1_ready),
                         qk_proj_gen(1, wk_sb, kt_sb, n), QT)
            for c in range(ntc):
                add_unit(('v1', c), max(x_ready[c // 4], wv1_ready),
                         v_proj_gen(1, c), 128)

            coll_done = {}      # (pair, qi) -> est completion ns

            # ================= attention =================
            def attention_qi(pair, qi, extra=(), pre_diag=None):
                """extra: list of (frac, kind, key) actions fired when the
                emission reaches `frac` of this qi's exp tiles. kind 'pump'
                force-emits a deferred event; kind 'unit' unlocks a filler
                unit (pumping its pulls first)."""
                g = 4 * qi
                q0 = qi * QT
                total_tiles = g + 3
                tcount = [0]
                pend_extra = list(extra)

                def poll_extra():
                    frac = tcount[0] / total_tiles
                    for ex in list(pend_extra):
                        if frac >= ex[0]:
                            if ex[1] == 'pump':
                                pump_force(ex[2])
                            else:
                                flush_pull()
                                if ex[2] in by_key:
                                    by_key[ex[2]].ready = 0.0
                            pend_extra.remove(ex)

                def qk_mm(dst, kb, qa, w, h01):
                    nc.tensor.matmul(
                        dst,
                        lhsT=kt_sb[h01 * 64:(h01 + 1) * 64,
                                   pair * t_len + kb * 128: pair * t_len + (kb + 1) * 128],
                        rhs=qt_sb[h01 * 64:(h01 + 1) * 64,
                                  pair * t_len + qa: pair * t_len + qa + w],
                        start=True, stop=True,
                        tile_position=(h01 * 64, 0))
                    pe(w)

                Yab = yps.tile([128, 260], f32, name="Yab", tag="Yab")
                Ycd = yps.tile([128, 260], f32, name="Ycd", tag="Ycd")
                Y = {0: (Yab, 0), 1: (Yab, 1), 2: (Ycd, 0), 3: (Ycd, 1)}
                started = set()
                stop_at = {0: ('b0', 0), 1: ('b0', 1),
                           2: ('b1', 0), 3: ('b1', 1)}

                def av_mm(e_tile, ecol, j, h01, kb, tag):
                    yt_, jj = Y[j]
                    # ONE start=True per Y tile per round: start marks the
                    # whole PSUM zero-region (bank) pending-zero, so each
                    # slot's first write then overwrites and later writes
                    # accumulate. A second start in the same bank would
                    # re-poison already-written slots.
                    key = id(yt_)
                    st_ = key not in started
                    started.add(key)
                    nc.tensor.matmul(
                        yt_[:, jj * 130 + h01 * 65: jj * 130 + (h01 + 1) * 65],
                        lhsT=e_tile[:, ecol: ecol + 128],
                        rhs=vhat_sb[:, kb * VW + (2 * pair + h01) * 65:
                                    kb * VW + (2 * pair + h01 + 1) * 65],
                        start=st_, stop=(stop_at[j] == tag),
                        skip_group_check=True)
                    pe(65)

                pend = None     # deferred AV list from the previous tile

                def flush_pend():
                    nonlocal pend
                    if pend is not None:
                        est[0] = max(est[0], pend[0])
                        for f in pend[1]:
                            f()
                        pend = None

                def stage_tile(e_tile, avs):
                    nonlocal pend
                    flush_pend()
                    tcount[0] += 1
                    poll_extra()
                    pump()
                    fill(max(0.0, (est[1] - est[0]) - 600.0))
                    pend = (est[1], [])
                    for a in avs:
                        pend[1].append(a)

                # full k-blocks
                for kb in range(g):
                    qk = qkps.tile([128, 1024], f32, name="qk", tag="qk")
                    for h01 in (0, 1):
                        qk_mm(qk[:, h01 * 512:(h01 + 1) * 512], kb, q0, 512, h01)
                    e = epool.tile([128, 1024], bf16, name="e")
                    nc.scalar.activation(e[:], qk[:],
                                         mybir.ActivationFunctionType.Exp,
                                         scale=1.0 / np.sqrt(HD))
                    act_tile()
                    avs = []
                    for h01 in (0, 1):
                        for j in range(4):
                            avs.append(
                                (lambda e_=e, h_=h01, j_=j, kb_=kb:
                                 av_mm(e_, h_ * 512 + j_ * 128, j_, h_, kb_,
                                       ('full', kb_))))
                    stage_tile(e, avs)

                # K tile qi / V chunks 4qi..4qi+3 are first needed here; a
                # late force keeps the head of the q-tile exp-dense
                if pre_diag is not None:
                    pre_diag()

                # diagonal: mid supertile (kb g,g+1 vs upper q-half, unmasked)
                mid = qkps.tile([128, 1024], f32, name="qk", tag="qk")
                for i in (0, 1):
                    for h01 in (0, 1):
                        qk_mm(mid[:, (h01 * 2 + i) * 256:(h01 * 2 + i + 1) * 256],
                              g + i, q0 + 256, 256, h01)
                em = epool.tile([128, 1024], bf16, name="e")
                nc.scalar.activation(em[:], mid[:],
                                     mybir.ActivationFunctionType.Exp,
                                     scale=1.0 / np.sqrt(HD))
                act_tile()
                avs = []
                for h01 in (0, 1):
                    for i in (0, 1):
                        for jj in (0, 1):   # j = 2 + jj
                            avs.append(
                                (lambda e_=em, h_=h01, i_=i, jj_=jj:
                                 av_mm(e_, (h_ * 2 + i_) * 256 + jj_ * 128,
                                       2 + jj_, h_, g + i_, ('mid', i_))))
                stage_tile(em, avs)

                # diagonal bands (masked): band u covers q-half u vs kb g+2u+{0,1}
                # band order (1, 0): Ycd (q-blocks 2,3) closes one tile early
                # so its normalize/ship overlaps band0's compute.
                yt = ytp.tile([128, QT], bf16, name="yt")
                ytT = yttp.tile([128, QT], bf16, name="ytT")

                def norm_half(jp, Yt):
                    """normalize y[q,0:64] /= y[q,64] for q-blocks 2jp,2jp+1
                    into yt (DVE only; shipping happens once per q-tile)."""
                    recip = rpool.tile([128, 4], f32, name="recip")
                    nc.vector.reciprocal(
                        recip[:].rearrange("p (j c) -> p j c", c=1),
                        Yt.rearrange("p (j c) -> p j c", c=65)[:, :, 64:65])
                    for jj in (0, 1):
                        j = 2 * jp + jj
                        for h01 in (0, 1):
                            nc.vector.tensor_scalar_mul(
                                yt[:, j * 128 + h01 * 64: j * 128 + (h01 + 1) * 64],
                                Yt[:, jj * 130 + h01 * 65: jj * 130 + h01 * 65 + 64],
                                recip[:, jj * 2 + h01: jj * 2 + h01 + 1])

                for u in (1, 0):
                    bd = qkps.tile([128, 1024], f32, name="qk", tag="qk")
                    for i in (0, 1):
                        for h01 in (0, 1):
                            qk_mm(bd[:, (h01 * 2 + i) * 256:(h01 * 2 + i + 1) * 256],
                                  g + 2 * u + i, q0 + u * 256, 256, h01)
                    eb = epool.tile([128, 1024], bf16, name="e")
                    nc.scalar.activation(eb[:], bd[:],
                                         mybir.ActivationFunctionType.Exp,
                                         scale=1.0 / np.sqrt(HD))
                    act_tile()
                    nc.vector.tensor_mul(eb[:], eb[:], mask_sb[:])
                    avs = []
                    for h01 in (0, 1):
                        for i in (0, 1):
                            for jj in (0, 1):
                                if i == 1 and jj == 0:
                                    continue    # fully masked-out slot
                                avs.append(
                                    (lambda e_=eb, h_=h01, i_=i, jj_=jj, u_=u:
                                     av_mm(e_, (h_ * 2 + i_) * 256 + jj_ * 128,
                                           2 * u_ + jj_, h_, g + 2 * u_ + i_,
                                           (f'b{u_}', i_))))
                    stage_tile(eb, avs)
                    if u == 0:
                        # band1's AVs were just flushed -> Ycd is closed
                        norm_half(1, Ycd)
                flush_pend()
                norm_half(0, Yab)
                # one transpose + one agin per q-tile: keeps the sync HWDGE
                # queues sparse so FIFO sem thresholds resolve promptly
                in_t, out_t, coff, fire, qis = ship_plan[(pair, qi)]
                nc.sync.dma_start_transpose(
                    ytT[:].rearrange("p (j q) -> p j q", j=4), yt[:])
                nc.sync.dma_start(in_t[:, coff:coff + QT], ytT[:])
                if fire:
                    gl = in_t.shape[1]
                    nc.gpsimd.collective_compute(
                        "AllGather", mybir.AluOpType.bypass,
                        replica_groups=GROUPS,
                        ins=[in_t.opt()], outs=[out_t.opt()])
                    done_t = max(est[0], est[1]) + 11000.0
                    # chained pulls: emit the PREVIOUS collective's pull now,
                    # just after dispatching this one. A pull parks Pool.SEQ
                    # until its collective completes; chaining keeps that park
                    # from delaying a collective dispatch, and keeps parked
                    # DMAs off the sync HWDGE queues (whose FIFO semaphores
                    # would delay every later DMA sharing the queue).
                    t = ygp.tile([128, 4 * gl], bf16, name="yg",
                                 tag=f"yg{gl}", bufs=(6 if gl == QT else 1))
                    for pq in qis:
                        coll_done[pq] = done_t
                        ygs[pq] = (t, gl, (pq[1] - qis[0][1]) * QT)
                    flush_pull()
                    pend_pull[0] = (t, out_t, gl)

            # ================= main schedule =================
            def make_pre_diag(tag_k, tag_v, qi):
                def f():
                    force((tag_k, qi))
                    for c in range(4 * qi, 4 * qi + 4):
                        force((tag_v, c))
                return f

            for qi in range(nqt):
                force(('q0', qi))
                for n in range(qi):
                    force(('k0', n))
                for c in range(4 * qi):
                    force(('v0', c))
                attention_qi(0, qi, pre_diag=make_pre_diag('k0', 'v0', qi))

            for qi in range(nqt):
                force(('q1', qi))
                for n in range(qi):
                    force(('k1', n))
                for c in range(4 * qi):
                    force(('v1', c))
                extra = []
                if qi == 3:
                    extra += [(0.05, 'unit', ('op', 0)),
                              (0.35, 'unit', ('op', 1)),
                              (0.85, 'unit', ('op', 2))]
                attention_qi(1, qi, extra,
                             pre_diag=make_pre_diag('k1', 'v1', qi))
                for tq in ([0, 1] if qi == 1 else [qi] if qi >= 2 else []):
                    add_unit(('op', tq),
                             max(coll_done[(0, tq)], coll_done[(1, tq)],
                                 wo_ready) + 1000.0,
                             o_proj_gen(tq), QT)

            flush_pull()
            for tq in range(nqt):
                force(('op', tq))
            # flush any remaining deferred events (out DMAs of the tail)
            for ev in events:
                if not ev[2]:
                    ev[1]()
                    ev[2] = True

    nc.compile()
    return nc


def _masks_np():
    """Diagonal causal mask: [ki, qi] = qi >= ki, duplicated along the free
    axis for the two packed heads."""
    ki = np.arange(128)[:, None]
    qi = np.arange(128)[None, :]
    tri = (qi >= ki).astype(np.float32)
    ones = np.ones((128, 128), np.float32)
    zeros = np.zeros((128, 128), np.float32)
    lo = np.concatenate([tri, ones], axis=1)    # lower k-block of a band
    hi = np.concatenate([zeros, tri], axis=1)   # upper k-block of a band
    return np.concatenate([lo, hi, lo, hi], axis=1).astype(BF16)  # [128, 1024]


def _block(a, w):
    """[C, w] -> [128, NCB*w] partition-blocked bf16."""
    return np.ascontiguousarray(
        a.reshape(NCB, 128, w).transpose(1, 0, 2).reshape(128, NCB * w)).astype(BF16)


def _prep_inputs(x, Wq, Wk, Wv, Wo, t_len):
    masks = _masks_np()
    in_maps = []
    for c in range(N_CORES):
        b, hg = divmod(c, 4)
        sl = slice(hg * SH, (hg + 1) * SH)
        in_maps.append({
            "xT": _block(x[b].T, t_len),
            "wqT": _block(Wq[sl, :].T, SH),
            "wkT": _block(Wk[sl, :].T, SH),
            "wvT": _block(Wv[sl, :].T, SH),
            "woT": _block(Wo[sl, :].T, SH),
            "masks": masks,
        })
    return in_maps


def _assemble(results, t_len):
    out = np.empty((B, t_len, C), dtype=np.float32)
    for c in range(N_CORES):
        b, hg = divmod(c, 4)
        out[b, :, hg * SH:(hg + 1) * SH] = results[c]["out"].T.astype(np.float32)
    return out


def get_nc(t_len=T):
    if t_len not in _CACHE:
        _CACHE[t_len] = _build(t_len)
    return _CACHE[t_len]


def kernel(x, Wq, Wk, Wv, Wo):
    from concourse import bass_utils
    x = np.asarray(x, dtype=np.float32)
    nc = get_nc(T)
    in_maps = _prep_inputs(x, np.asarray(Wq), np.asarray(Wk), np.asarray(Wv),
                           np.asarray(Wo), T)
    res = bass_utils.run_bass_kernel_spmd(nc, in_maps, core_ids=list(range(N_CORES)))
    return _assemble(res.results, T)
